# revision 1
# baseline (speedup 1.0000x reference)
"""CWT (complex Morlet wavelet) transform kernel for Trainium2, 8 NeuronCores.

Math (mirrors the reference):
    sig = x.reshape(12, 16384), reflect-padded by 381 on both sides
    re/im = conv1d(sig, weight_real/imag)   # 128 filters, 763 taps
    mag = log1p(sqrt(re^2 + im^2 + 1e-8))
    out = mean-pool(mag, 64) -> (4, 3, 128, 256)

Device strategy (per core, time-sharded 8 ways -> 2048 output samples each):
  - im2col by shifted replication: SBUF tile shift[i, u] = sigpad[c*2048 + i + u]
    built with one overlapping-read DMA per signal.
  - conv as 6 accumulating matmuls per (signal, 512-tile, re/im):
    out[s, t] += wT_j[i, s]^T-contracted-with shift[:, t0+128j : +512], bf16 PE,
    fp32 PSUM. 128 scales live on PSUM partitions.
  - postproc: sq_re on DVE, sq_im on ACT(Square), add on DVE, then
    ln/exp/ln on ACT (sqrt(s) = exp(0.5*ln(s)) keeps a single ACT table set:
    natural_log_exp_and_others has ln+exp+square), mean-pool on DVE pool_avg.
"""

import numpy as np
import ml_dtypes

import concourse.bass as bass
import concourse.tile as tile
import concourse.mybir as mybir
from concourse.vector_clock import ScopedClock
from concourse.bass_utils import run_bass_kernel_spmd

N_CORES = 8
NSIG = 12            # B*C
T = 16384
TCHUNK = T // N_CORES          # 2048 output samples per core
TILE_N = 512                   # matmul free dim / postproc tile
NT0 = TCHUNK // TILE_N         # 4 tiles per core
KTAPS = 763
KPAD = 768                     # 6 blocks of 128
NBLK = 6
PAD = KTAPS // 2               # 381
U = (NT0 - 1) * TILE_N + (NBLK - 1) * 128 + TILE_N    # 2688 shift columns
SIG_ROW = U + 128              # 2816 elements per core per signal
BF16 = mybir.dt.bfloat16
F32 = mybir.dt.float32


class _TC(tile.TileContext):
    """TileContext whose final drain carries no sem waits.

    The walrus build in this container rejects any sync-wait commands
    attached to SP CTRL instructions (Drain/NoOp): "Too many sync wait
    commands".  Split the frontier waits into one single-wait NOP each,
    then emit a bare drain.
    """

    def _drain_and_barrier(self, tick_clock, wait_clock):
        nop_inst = self.nc.sync.nop(nofuse=True)
        wait_clock.add_sem_waits(
            nop_inst.ins, ScopedClock({None: tick_clock.global_clock})
        )
        si = nop_inst.ins.sync_info
        waits = list(si.on_wait) if si else []
        while si is not None and si.on_wait:
            si.on_wait.pop()
        for w in waits:
            wi = self.nc.sync.nop(nofuse=True)
            wi.ins.sync_info = mybir.SyncInfo(on_update=[], on_wait=[w])
        self.nc.sync.drain()
        self.nc.all_engine_barrier()
        assert self.sems is not None
        popped = self.nc._tile_sem_poison_stack.pop()
        assert popped is self._sem_poison
        self.nc.clear_and_free_semaphores(list(self.sems.allocated().values()))
        self.nc.all_engine_barrier()


def _split_sync_waits(nc):
    """Hoist sync waits onto single-wait NOPs.

    The walrus build here accepts at most ONE sync-wait command per
    instruction (and none on Drain).  Engine instruction streams execute
    in order, so a NOP on the same engine carrying the extra waits,
    emitted immediately before the real instruction, is equivalent.
    """
    n = 0
    for fn in nc.m.functions:
        for bb in fn.blocks:
            new = []
            changed = False
            for inst in bb.instructions:
                si = getattr(inst, "sync_info", None)
                waits = list(si.on_wait) if si is not None and si.on_wait else []
                budget = 0 if inst.opcode == "Drain" else 1
                if len(waits) > budget:
                    keep = waits[len(waits) - budget :] if budget else []
                    extra = waits[: len(waits) - budget]
                    for w in extra:
                        n += 1
                        new.append(
                            mybir.InstNoOp(
                                name=f"I-wsplit-{n}",
                                engine=inst.engine,
                                ins=[],
                                outs=[],
                                sync_info=mybir.SyncInfo(on_wait=[w], on_update=[]),
                            )
                        )
                    inst.sync_info = mybir.SyncInfo(
                        on_wait=keep, on_update=list(si.on_update)
                    )
                    changed = True
                new.append(inst)
            if changed:
                bb.instructions = new
    return n


def build_program(n_sig=NSIG, n_t0=NT0, hop=64, split_waits=True):
    """Build the per-core Bass program (identical for all 8 cores)."""
    assert TILE_N % hop == 0
    fpt = TILE_N // hop           # frames per 512-tile (8 for hop=64)
    nframes = n_t0 * fpt          # frames per core per signal (32)

    nc = bass.Bass()
    # register the ln-bias constant (only 0.0/1.0 are pre-registered)
    _eps_t = nc.alloc_sbuf_tensor("const-float32-eps", [128, 1], F32)
    nc.gpsimd.memset(_eps_t.ap(), 1e-8)
    nc.const_aps.aps[(F32, 1e-8)] = _eps_t.ap()
    nc.all_engine_barrier()

    sig_d = nc.dram_tensor("sig", [n_sig, SIG_ROW], BF16, kind="ExternalInput")
    wt_d = nc.dram_tensor("wt", [128, 2, NBLK, 128], BF16, kind="ExternalInput")
    out_d = nc.dram_tensor("out", [n_sig, 128, nframes], F32, kind="ExternalOutput")

    AF = mybir.ActivationFunctionType

    with _TC(nc) as tc:
        with (
            tc.tile_pool(name="singles", bufs=1) as singles,
            tc.tile_pool(name="psum", bufs=2, space="PSUM") as psum,
            tc.tile_pool(name="post", bufs=3) as post,
            tc.tile_pool(name="outp", bufs=2) as outp,
        ):
            # weights: [taps_i, cplx, block_j, scales]
            wts = singles.tile([128, 2, NBLK, 128], BF16, tag="wts")
            nc.sync.dma_start(wts[:], wt_d[:])

            # shifted-replica tiles, one per signal
            base = sig_d[:]
            shifts = []
            for s in range(n_sig):
                sh = singles.tile([128, U], BF16, tag=f"shift{s}")
                src = bass.AP(
                    tensor=base.tensor,
                    offset=base.offset + s * SIG_ROW,
                    ap=[[1, 128], [1, U]],
                )
                nc.sync.dma_start(sh[:], src)
                shifts.append(sh)

            for s in range(n_sig):
                sh = shifts[s]
                osb = outp.tile([128, nframes], F32, tag="osb")
                for it in range(n_t0):
                    t0 = it * TILE_N
                    ps_re = psum.tile([128, TILE_N], F32, tag="re")
                    ps_im = psum.tile([128, TILE_N], F32, tag="im")
                    for j in range(NBLK):
                        nc.tensor.matmul(
                            ps_re[:],
                            lhsT=wts[:, 0, j, :],
                            rhs=sh[:, t0 + 128 * j : t0 + 128 * j + TILE_N],
                            start=(j == 0),
                            stop=(j == NBLK - 1),
                        )
                    for j in range(NBLK):
                        nc.tensor.matmul(
                            ps_im[:],
                            lhsT=wts[:, 1, j, :],
                            rhs=sh[:, t0 + 128 * j : t0 + 128 * j + TILE_N],
                            start=(j == 0),
                            stop=(j == NBLK - 1),
                        )
                    # walrus here rejects TT with both operands in PSUM, so
                    # evacuate re via copy and square in SBUF; im squares on ACT
                    cre = post.tile([128, TILE_N], BF16, tag="cre")
                    nc.vector.tensor_copy(cre[:], ps_re[:])
                    sq_im = post.tile([128, TILE_N], BF16, tag="sqim")
                    nc.scalar.activation(sq_im[:], ps_im[:], AF.Square)
                    sq_re = post.tile([128, TILE_N], BF16, tag="sqre")
                    nc.vector.tensor_mul(sq_re[:], cre[:], cre[:])
                    ssum = post.tile([128, TILE_N], BF16, tag="ssum")
                    nc.vector.tensor_add(ssum[:], sq_re[:], sq_im[:])
                    # ln(s + 1e-8) in fp16 (bf16 would wreck exp(0.5*u))
                    u = post.tile([128, TILE_N], mybir.dt.float16, tag="u")
                    nc.scalar.activation(u[:], ssum[:], AF.Ln, bias=1e-8)
                    # v = exp(u/2) = sqrt(s + 1e-8)
                    v = post.tile([128, TILE_N], BF16, tag="v")
                    nc.scalar.activation(v[:], u[:], AF.Exp, scale=0.5)
                    # l = ln(1 + v)
                    l = post.tile([128, TILE_N], BF16, tag="l")
                    nc.scalar.activation(l[:], v[:], AF.Ln, bias=1.0)
                    nc.vector.tensor_reduce(
                        osb[:, it * fpt : (it + 1) * fpt],
                        l[:].rearrange("p (f w) -> p f w", w=hop),
                        axis=mybir.AxisListType.X,
                        op=mybir.AluOpType.add,
                    )
                # mean = sum / hop
                nc.scalar.mul(osb[:], osb[:], 1.0 / hop)
                nc.sync.dma_start(out_d[s], osb[:])
    if split_waits:
        _split_sync_waits(nc)
    return nc


def prep_inputs(x, weight_real, weight_imag, hop):
    """Host-side shard/layout prep. Returns per-core input maps."""
    x = np.asarray(x, dtype=np.float32)
    wr = np.asarray(weight_real, dtype=np.float32)
    wi = np.asarray(weight_imag, dtype=np.float32)
    B, C, _ = x.shape

    sig = x.reshape(B * C, T)
    sigpad = np.pad(sig, ((0, 0), (PAD, PAD)), mode="reflect")
    total = (N_CORES - 1) * TCHUNK + SIG_ROW          # 17152
    sigpad = np.pad(sigpad, ((0, 0), (0, total - sigpad.shape[1])))
    sig_bf = sigpad.astype(ml_dtypes.bfloat16)

    # weights -> [taps_i, cplx, block_j, scales] bf16, zero-padded to 768 taps
    wpad = np.zeros((2, 128, KPAD), np.float32)
    wpad[0, :, :KTAPS] = wr[:, 0, :]
    wpad[1, :, :KTAPS] = wi[:, 0, :]
    # (c, s, j, i) -> (i, c, j, s)
    wt_host = np.ascontiguousarray(
        wpad.reshape(2, 128, NBLK, 128).transpose(3, 0, 2, 1)
    ).astype(ml_dtypes.bfloat16)

    in_maps = []
    for c in range(N_CORES):
        chunk = np.ascontiguousarray(sig_bf[:, c * TCHUNK : c * TCHUNK + SIG_ROW])
        in_maps.append({"sig": chunk, "wt": wt_host})
    return in_maps


def prep_wt2(weight_real, weight_imag):
    wr = np.asarray(weight_real, dtype=np.float32)
    wi = np.asarray(weight_imag, dtype=np.float32)
    wpad = np.zeros((2, 128, KPAD), np.float32)
    wpad[0, :, :KTAPS] = wr[:, 0, :]
    wpad[1, :, :KTAPS] = wi[:, 0, :]
    # wt2[i, j, 2s+c] = wpad[c, s, 128j+i]
    w4 = wpad.reshape(2, 128, NBLK, 128)          # (c, s, j, i)
    wt2 = np.ascontiguousarray(w4.transpose(3, 2, 1, 0).reshape(128, NBLK, 256))
    return wt2.astype(ml_dtypes.bfloat16)


def prep_pmat(n_tiles=16, hop=64):
    """Pooling matrices: P[t, it, f] = 1/hop if f == fpt*it + t//hop."""
    fpt = 128 // hop
    nframes = n_tiles * fpt
    P = np.zeros((128, n_tiles, nframes), np.float32)
    for it in range(n_tiles):
        for t in range(128):
            P[t, it, fpt * it + t // hop] = 1.0 / hop
    return P.astype(ml_dtypes.bfloat16)


#  scale-support prefix: block j of 128 taps is needed only by the first S_j
#  scales (supports shrink monotonically with scale index).  Computed from
#  the morlet construction: th_s = int(7639.44/f_s), block j needed iff
#  [381-th, 381+th] overlaps [128j, 128j+128).
S_J = [17, 46, 128, 128, 44, 16]
J_ORDER = [2, 3, 1, 4, 0, 5]          # S_j descending


def build_program_v2(n_sig=NSIG, n_tiles=16, hop=64, act_square_every=5,
                     split_waits=True):
    """Transposed conv: time on PSUM partitions, scales streamed (sparse).

    Per (signal, 128-sample tile): 6 matmuls, lhsT = shift slice
    [128 taps, 128 t], rhs = weights [128 taps, (2cplx, S_j scales)],
    accumulated into psum [128 t, 2, 128].  Streams 758 columns per tile
    instead of 1536 (支持 prefix sparsity).  Postproc: squares split
    ACT/DVE, adds on GPSIMD into s_sig [128, n_tiles*128], then a
    per-signal Ln/Exp/Ln chain (single ACT table set), then pooling as
    16 accumulating matmuls with a [128, 32] pooling matrix ->
    psum [32 frames, 128 scales] -> DRAM [n_sig, 32, 128].
    """
    assert hop == 64, "v2 pooling matrices assume hop=64"
    fpt = 128 // hop                      # frames per 128-tile (2)
    nframes = n_tiles * fpt               # 32
    TW = n_tiles * 128                    # 2048 time samples per signal

    nc = bass.Bass()
    _eps_t = nc.alloc_sbuf_tensor("const-float32-eps", [128, 1], F32)
    nc.gpsimd.memset(_eps_t.ap(), 1e-8)
    nc.const_aps.aps[(F32, 1e-8)] = _eps_t.ap()
    nc.all_engine_barrier()

    sig_d = nc.dram_tensor("sig", [n_sig, SIG_ROW], BF16, kind="ExternalInput")
    # weights interleaved (scale, cplx): col 2s+c, so每 block's rhs and psum
    # writes are contiguous prefixes [0, 2*S_j)
    wt_d = nc.dram_tensor("wt2", [128, NBLK, 256], BF16, kind="ExternalInput")
    pmat_d = nc.dram_tensor("pmat", [128, n_tiles, nframes], BF16,
                            kind="ExternalInput")
    out_d = nc.dram_tensor("out", [n_sig, nframes, 128], F32,
                           kind="ExternalOutput")

    AF = mybir.ActivationFunctionType

    with _TC(nc) as tc:
        with (
            tc.tile_pool(name="singles", bufs=1) as singles,
            tc.tile_pool(name="psum", bufs=4, space="PSUM") as psum,
            tc.tile_pool(name="post", bufs=4) as post,
            tc.tile_pool(name="sigbuf", bufs=2) as sigbuf,
            tc.tile_pool(name="outp", bufs=2) as outp,
        ):
            wts = singles.tile([128, NBLK, 256], BF16, tag="wts")
            nc.sync.dma_start(wts[:], wt_d[:])
            pmat = singles.tile([128, n_tiles, nframes], BF16, tag="pmat")
            nc.sync.dma_start(pmat[:], pmat_d[:])

            base = sig_d[:]
            shifts = []
            for s in range(n_sig):
                sh = singles.tile([128, U], BF16, tag=f"shift{s}")
                src = bass.AP(
                    tensor=base.tensor,
                    offset=base.offset + s * SIG_ROW,
                    ap=[[1, 128], [1, U]],
                )
                nc.sync.dma_start(sh[:], src)
                shifts.append(sh)

            tilectr = 0
            for s in range(n_sig):
                sh = shifts[s]
                s_sig = sigbuf.tile([128, TW], BF16, tag="s_sig")
                for it in range(n_tiles):
                    ps = psum.tile([128, 256], F32, tag="conv")
                    for k, j in enumerate(J_ORDER):
                        sj = S_J[j]
                        nc.tensor.matmul(
                            ps[:, 0 : 2 * sj],
                            lhsT=sh[:, 128 * (it + j) : 128 * (it + j) + 128],
                            rhs=wts[:, j, 0 : 2 * sj],
                            start=(k == 0),
                            stop=(k == NBLK - 1),
                            skip_group_check=True,
                        )
                    # squares: rotate a fraction onto ACT, rest on DVE
                    sqb = post.tile([128, 256], BF16, tag="sqb")
                    if tilectr % act_square_every == 0:
                        nc.scalar.activation(sqb[:], ps[:], AF.Square)
                    else:
                        cb = post.tile([128, 256], BF16, tag="cb")
                        nc.vector.tensor_copy(cb[:], ps[:])
                        nc.vector.tensor_mul(sqb[:], cb[:], cb[:])
                    tilectr += 1
                    sq3 = sqb[:].rearrange("p (s c) -> p s c", c=2)
                    nc.gpsimd.tensor_tensor(
                        s_sig[:, it * 128 : (it + 1) * 128],
                        sq3[:, :, 0],
                        sq3[:, :, 1],
                        mybir.AluOpType.add,
                    )
                # ln/exp/ln chain over the whole signal (one ACT table set)
                u = sigbuf.tile([128, TW], mybir.dt.float16, tag="u")
                nc.scalar.activation(u[:], s_sig[:], AF.Ln, bias=1e-8)
                v = sigbuf.tile([128, TW], BF16, tag="v")
                nc.scalar.activation(v[:], u[:], AF.Exp, scale=0.5)
                l = sigbuf.tile([128, TW], BF16, tag="l")
                nc.scalar.activation(l[:], v[:], AF.Ln, bias=1.0)
                # pooling: 16 accumulating matmuls -> [32 frames, 128 scales]
                pps = psum.tile([nframes, 128], F32, tag="pool", bufs=2)
                for it in range(n_tiles):
                    nc.tensor.matmul(
                        pps[:],
                        lhsT=pmat[:, it, :],
                        rhs=l[:, it * 128 : (it + 1) * 128],
                        start=(it == 0),
                        stop=(it == n_tiles - 1),
                        skip_group_check=True,
                    )
                osb = outp.tile([nframes, 128], F32, tag="osb")
                nc.vector.tensor_copy(osb[:], pps[:])
                nc.sync.dma_start(out_d[s], osb[:])
    if split_waits:
        _split_sync_waits(nc)
    return nc


def build_program_v3(n_sig=NSIG, hop=64, n_a=6, split_waits=True,
                     no_pool_b=False, act_squares=3, interleave=True):
    """Hybrid: half the signals conv'd weights-stationary (v1 layout, MM-stream
    heavy), half signal-stationary with scale-prefix sparsity (v2 layout,
    LDW-stream heavy).  The PE's LDWEIGHTS path (1.2 GHz) and matmul column
    stream (2.4 GHz) are parallel resources; interleaving the two forms
    balances them at ~95us instead of 123us for either alone.
    Postproc for both layouts: squares split ACT/DVE, adds on GPSIMD into
    s_sig [128, 2048] bf16, per-signal Ln/Exp/Ln chain (single table set),
    pooling: DVE grouped reduce (A/scale-major) or PE matmul (B/time-major).
    """
    assert hop == 64
    n_b = n_sig - n_a
    NT128 = TCHUNK // 128                 # 16 128-tiles per signal (B form)
    fpt512 = TILE_N // hop                # 8 frames per 512-tile (A form)
    nframes = TCHUNK // hop               # 32

    nc = bass.Bass()
    _eps_t = nc.alloc_sbuf_tensor("const-float32-eps", [128, 1], F32)
    nc.gpsimd.memset(_eps_t.ap(), 1e-8)
    nc.const_aps.aps[(F32, 1e-8)] = _eps_t.ap()
    nc.all_engine_barrier()

    sig_d = nc.dram_tensor("sig", [n_sig, SIG_ROW], BF16, kind="ExternalInput")
    wt_d = nc.dram_tensor("wt", [128, 2, NBLK, 128], BF16, kind="ExternalInput")
    wt2_d = nc.dram_tensor("wt2", [128, NBLK, 256], BF16, kind="ExternalInput")
    pmat_d = nc.dram_tensor("pmat", [128, NT128, nframes], BF16,
                            kind="ExternalInput")
    outa_d = nc.dram_tensor("outa", [max(n_a, 1), 128, nframes], F32,
                            kind="ExternalOutput")
    outb_d = nc.dram_tensor("outb", [max(n_b, 1), nframes, 128], F32,
                            kind="ExternalOutput")

    AF = mybir.ActivationFunctionType

    with _TC(nc) as tc:
        with (
            tc.tile_pool(name="singles", bufs=1) as singles,
            tc.tile_pool(name="psum", bufs=2, space="PSUM") as psum,
            tc.tile_pool(name="post", bufs=4) as post,
            tc.tile_pool(name="sigbuf", bufs=2) as sigbuf,
            tc.tile_pool(name="outp", bufs=2) as outp,
        ):
            wts = singles.tile([128, 2, NBLK, 128], BF16, tag="wts")
            nc.sync.dma_start(wts[:], wt_d[:])
            wts2 = singles.tile([128, NBLK, 256], BF16, tag="wts2")
            nc.sync.dma_start(wts2[:], wt2_d[:])
            pmat = singles.tile([128, NT128, nframes], BF16, tag="pmat")
            nc.sync.dma_start(pmat[:], pmat_d[:])

            base = sig_d[:]
            shifts = []
            for s in range(n_sig):
                sh = singles.tile([128, U], BF16, tag=f"shift{s}")
                src = bass.AP(
                    tensor=base.tensor,
                    offset=base.offset + s * SIG_ROW,
                    ap=[[1, 128], [1, U]],
                )
                nc.sync.dma_start(sh[:], src)
                shifts.append(sh)

            tilectr = 0

            def conv_a_tile(sh, s_sig, it):
                nonlocal tilectr
                t0 = it * TILE_N
                ps_re = psum.tile([128, TILE_N], F32, tag="are", name="are")
                ps_im = psum.tile([128, TILE_N], F32, tag="aim", name="aim")
                for j in range(NBLK):
                    nc.tensor.matmul(
                        ps_re[:], lhsT=wts[:, 0, j, :],
                        rhs=sh[:, t0 + 128 * j : t0 + 128 * j + TILE_N],
                        start=(j == 0), stop=(j == NBLK - 1),
                        skip_group_check=True,
                    )
                for j in range(NBLK):
                    nc.tensor.matmul(
                        ps_im[:], lhsT=wts[:, 1, j, :],
                        rhs=sh[:, t0 + 128 * j : t0 + 128 * j + TILE_N],
                        start=(j == 0), stop=(j == NBLK - 1),
                        skip_group_check=True,
                    )
                # squares -> s_sig[:, t0:t0+512]
                sq_re = post.tile([128, TILE_N], BF16, tag="asqre", name="asqre")
                if act_squares and tilectr % act_squares == 0:
                    nc.scalar.activation(sq_re[:], ps_re[:], AF.Square)
                else:
                    cre = post.tile([128, TILE_N], BF16, tag="acre", name="acre")
                    nc.vector.tensor_copy(cre[:], ps_re[:])
                    nc.vector.tensor_mul(sq_re[:], cre[:], cre[:])
                sq_im = post.tile([128, TILE_N], BF16, tag="asqim", name="asqim")
                if act_squares and tilectr % act_squares == 1:
                    nc.scalar.activation(sq_im[:], ps_im[:], AF.Square)
                else:
                    cim = post.tile([128, TILE_N], BF16, tag="acim", name="acim")
                    nc.vector.tensor_copy(cim[:], ps_im[:])
                    nc.vector.tensor_mul(sq_im[:], cim[:], cim[:])
                tilectr += 1
                nc.gpsimd.tensor_tensor(
                    s_sig[:, t0 : t0 + TILE_N], sq_re[:], sq_im[:],
                    mybir.AluOpType.add,
                )

            def conv_b_tile(sh, s_sig, it):
                nonlocal tilectr
                ps = psum.tile([128, 256], F32, tag="bconv", name="bconv")
                for k, j in enumerate(J_ORDER):
                    sj = S_J[j]
                    nc.tensor.matmul(
                        ps[:, 0 : 2 * sj],
                        lhsT=sh[:, 128 * (it + j) : 128 * (it + j) + 128],
                        rhs=wts2[:, j, 0 : 2 * sj],
                        start=(k == 0), stop=(k == NBLK - 1),
                        skip_group_check=True,
                    )
                sqb = post.tile([128, 256], BF16, tag="bsqb", name="bsqb")
                if act_squares and tilectr % act_squares == 0:
                    nc.scalar.activation(sqb[:], ps[:], AF.Square)
                else:
                    cb = post.tile([128, 256], BF16, tag="bcb", name="bcb")
                    nc.vector.tensor_copy(cb[:], ps[:])
                    nc.vector.tensor_mul(sqb[:], cb[:], cb[:])
                tilectr += 1
                sq3 = sqb[:].rearrange("p (s c) -> p s c", c=2)
                nc.gpsimd.tensor_tensor(
                    s_sig[:, it * 128 : (it + 1) * 128],
                    sq3[:, :, 0], sq3[:, :, 1], mybir.AluOpType.add,
                )

            def chain(s_sig):
                u = sigbuf.tile([128, TCHUNK], mybir.dt.float16, tag="u", name="u")
                nc.scalar.activation(u[:], s_sig[:], AF.Ln, bias=1e-8)
                v = sigbuf.tile([128, TCHUNK], BF16, tag="v", name="v")
                nc.scalar.activation(v[:], u[:], AF.Exp, scale=0.5)
                l = sigbuf.tile([128, TCHUNK], BF16, tag="l", name="l")
                nc.scalar.activation(l[:], v[:], AF.Ln, bias=1.0)
                return l

            def finish_a(l, sa):
                osb = outp.tile([128, nframes], F32, tag="osba", name="osba")
                nc.vector.tensor_reduce(
                    osb[:], l[:].rearrange("p (f w) -> p f w", w=hop),
                    axis=mybir.AxisListType.X, op=mybir.AluOpType.add,
                )
                nc.scalar.mul(osb[:], osb[:], 1.0 / hop)
                nc.sync.dma_start(outa_d[sa], osb[:])

            def finish_b(l, sb):
                osb = outp.tile([nframes, 128], F32, tag="osbb", name="osbb")
                if no_pool_b:
                    nc.vector.tensor_copy(osb[:], l[:, 0:nframes].rearrange("p f -> p f"))
                    nc.sync.dma_start(outb_d[sb], osb[:].rearrange("p f -> p f"))
                    return
                pps = psum.tile([nframes, 128], F32, tag="bpool", name="bpool")
                for it in range(NT128):
                    nc.tensor.matmul(
                        pps[:], lhsT=pmat[:, it, :],
                        rhs=l[:, it * 128 : (it + 1) * 128],
                        start=(it == 0), stop=(it == NT128 - 1),
                        skip_group_check=True,
                    )
                nc.vector.tensor_copy(osb[:], pps[:])
                nc.sync.dma_start(outb_d[sb], osb[:])

            # interleave A and B signals pairwise so both PE streams stay busy
            npairs = max(n_a, n_b)
            for p in range(npairs):
                sa = p if p < n_a else None
                sb = p if p < n_b else None
                ssa = (
                    sigbuf.tile([128, TCHUNK], BF16, tag="ssa", name="ssa")
                    if sa is not None else None
                )
                ssb = (
                    sigbuf.tile([128, TCHUNK], BF16, tag="ssb", name="ssb")
                    if sb is not None else None
                )
                if interleave:
                    for k in range(NT0):      # 4 super-steps
                        if sa is not None:
                            conv_a_tile(shifts[sa], ssa, k)
                        if sb is not None:
                            for it in range(4 * k, 4 * k + 4):
                                conv_b_tile(shifts[n_a + sb], ssb, it)
                else:
                    if sa is not None:
                        for k in range(NT0):
                            conv_a_tile(shifts[sa], ssa, k)
                    if sb is not None:
                        for it in range(NT128):
                            conv_b_tile(shifts[n_a + sb], ssb, it)
                if sa is not None:
                    finish_a(chain(ssa), sa)
                if sb is not None:
                    finish_b(chain(ssb), sb)
    if split_waits:
        _split_sync_waits(nc)
    return nc


def _ensure_ntff_hook():
    """Provide antenv.axon_hooks (missing in this image) so trace=True works."""
    import sys as _sys
    import types as _types

    try:
        from antenv.axon_hooks import get_axon_ntff_profile_hook  # noqa: F401
        return
    except ImportError:
        pass
    import antenv
    from trn_agent_boot.trn_boot import _ntff_profile_via_ctypes

    mod = _types.ModuleType("antenv.axon_hooks")
    holder = [None]
    mod.set_axon_ntff_profile_hook = lambda h: holder.__setitem__(0, h)
    mod.get_axon_ntff_profile_hook = lambda: holder[0]
    _sys.modules["antenv.axon_hooks"] = mod
    antenv.axon_hooks = mod
    mod.set_axon_ntff_profile_hook(
        _ntff_profile_via_ctypes("/opt/axon/libaxon_pjrt.so")
    )


_prog_cache = {}


def run(x, weight_real, weight_imag, hop_length, trace=False, trace_kwargs=None,
        version=2):
    """Run the kernel on 8 cores; returns (output, BassKernelResults)."""
    hop = int(hop_length)
    key = (version, hop)
    if key not in _prog_cache:
        if version == 3 and hop == 64:
            _prog_cache[key] = build_program_v3(hop=hop)
        elif version == 2 and hop == 64:
            _prog_cache[key] = build_program_v2(hop=hop)
        else:
            key = (1, hop)
            if key not in _prog_cache:
                _prog_cache[key] = build_program(hop=hop)
    nc = _prog_cache[key]
    version = key[0]

    in_maps = prep_inputs(x, weight_real, weight_imag, hop)
    if version in (2, 3):
        pmat = prep_pmat()
        wt2 = prep_wt2(weight_real, weight_imag)
        for m in in_maps:
            m["pmat"] = pmat
            m["wt2"] = wt2
            if version == 2:
                del m["wt"]
    kwargs = {}
    if trace:
        _ensure_ntff_hook()
        kwargs["trace"] = True
        kwargs.update(trace_kwargs or {})
    res = run_bass_kernel_spmd(nc, in_maps, core_ids=list(range(N_CORES)), **kwargs)

    B, C = 4, 3
    nf_core = TCHUNK // hop
    N_A = 6
    out = np.empty((NSIG, 128, N_CORES * nf_core), np.float32)
    for c in range(N_CORES):
        sl = slice(c * nf_core, (c + 1) * nf_core)
        if version == 3:
            out[:N_A, :, sl] = res.results[c]["outa"]
            out[N_A:, :, sl] = res.results[c]["outb"].transpose(0, 2, 1)
        elif version == 2:
            out[:, :, sl] = res.results[c]["out"].transpose(0, 2, 1)
        else:
            out[:, :, sl] = res.results[c]["out"]
    return out.reshape(B, C, 128, N_CORES * nf_core), res


def kernel(x, weight_real, weight_imag, hop_length):
    out, _ = run(x, weight_real, weight_imag, hop_length)
    return out



# revision 2
# speedup vs baseline: 1.2897x; 1.2897x over previous
"""CWT (complex Morlet wavelet) transform kernel for Trainium2, 8 NeuronCores.

Math (mirrors the reference):
    sig = x.reshape(12, 16384), reflect-padded by 381 on both sides
    re/im = conv1d(sig, weight_real/imag)   # 128 filters, 763 taps
    mag = log1p(sqrt(re^2 + im^2 + 1e-8))
    out = mean-pool(mag, 64) -> (4, 3, 128, 256)

Device strategy (per core, time-sharded 8 ways -> 2048 output samples each):
  - im2col by shifted replication: SBUF tile shift[i, u] = sigpad[c*2048 + i + u]
    built with one overlapping-read DMA per signal.
  - conv as 6 accumulating matmuls per (signal, 512-tile, re/im):
    out[s, t] += wT_j[i, s]^T-contracted-with shift[:, t0+128j : +512], bf16 PE,
    fp32 PSUM. 128 scales live on PSUM partitions.
  - postproc: sq_re on DVE, sq_im on ACT(Square), add on DVE, then
    ln/exp/ln on ACT (sqrt(s) = exp(0.5*ln(s)) keeps a single ACT table set:
    natural_log_exp_and_others has ln+exp+square), mean-pool on DVE pool_avg.
"""

import numpy as np
import ml_dtypes

import concourse.bass as bass
import concourse.tile as tile
import concourse.mybir as mybir
from concourse.vector_clock import ScopedClock
from concourse.bass_utils import run_bass_kernel_spmd

N_CORES = 8
NSIG = 12            # B*C
T = 16384
TCHUNK = T // N_CORES          # 2048 output samples per core
TILE_N = 512                   # matmul free dim / postproc tile
NT0 = TCHUNK // TILE_N         # 4 tiles per core
KTAPS = 763
KPAD = 768                     # 6 blocks of 128
NBLK = 6
PAD = KTAPS // 2               # 381
U = (NT0 - 1) * TILE_N + (NBLK - 1) * 128 + TILE_N    # 2688 shift columns
SIG_ROW = U + 128              # 2816 elements per core per signal
BF16 = mybir.dt.bfloat16
F32 = mybir.dt.float32


class _TC(tile.TileContext):
    """TileContext whose final drain carries no sem waits.

    The walrus build in this container rejects any sync-wait commands
    attached to SP CTRL instructions (Drain/NoOp): "Too many sync wait
    commands".  Split the frontier waits into one single-wait NOP each,
    then emit a bare drain.
    """

    def _drain_and_barrier(self, tick_clock, wait_clock):
        nop_inst = self.nc.sync.nop(nofuse=True)
        wait_clock.add_sem_waits(
            nop_inst.ins, ScopedClock({None: tick_clock.global_clock})
        )
        si = nop_inst.ins.sync_info
        waits = list(si.on_wait) if si else []
        while si is not None and si.on_wait:
            si.on_wait.pop()
        for w in waits:
            wi = self.nc.sync.nop(nofuse=True)
            wi.ins.sync_info = mybir.SyncInfo(on_update=[], on_wait=[w])
        self.nc.sync.drain()
        self.nc.all_engine_barrier()
        assert self.sems is not None
        popped = self.nc._tile_sem_poison_stack.pop()
        assert popped is self._sem_poison
        self.nc.clear_and_free_semaphores(list(self.sems.allocated().values()))
        self.nc.all_engine_barrier()


def _split_sync_waits(nc):
    """Hoist sync waits onto single-wait NOPs.

    The walrus build here accepts at most ONE sync-wait command per
    instruction (and none on Drain).  Engine instruction streams execute
    in order, so a NOP on the same engine carrying the extra waits,
    emitted immediately before the real instruction, is equivalent.
    """
    n = 0
    for fn in nc.m.functions:
        for bb in fn.blocks:
            new = []
            changed = False
            for inst in bb.instructions:
                si = getattr(inst, "sync_info", None)
                waits = list(si.on_wait) if si is not None and si.on_wait else []
                budget = 0 if inst.opcode == "Drain" else 1
                if len(waits) > budget:
                    keep = waits[len(waits) - budget :] if budget else []
                    extra = waits[: len(waits) - budget]
                    for w in extra:
                        n += 1
                        new.append(
                            mybir.InstNoOp(
                                name=f"I-wsplit-{n}",
                                engine=inst.engine,
                                ins=[],
                                outs=[],
                                sync_info=mybir.SyncInfo(on_wait=[w], on_update=[]),
                            )
                        )
                    inst.sync_info = mybir.SyncInfo(
                        on_wait=keep, on_update=list(si.on_update)
                    )
                    changed = True
                new.append(inst)
            if changed:
                bb.instructions = new
    return n


def build_program(n_sig=NSIG, n_t0=NT0, hop=64, split_waits=True):
    """Build the per-core Bass program (identical for all 8 cores)."""
    assert TILE_N % hop == 0
    fpt = TILE_N // hop           # frames per 512-tile (8 for hop=64)
    nframes = n_t0 * fpt          # frames per core per signal (32)

    nc = bass.Bass()
    # register the ln-bias constant (only 0.0/1.0 are pre-registered)
    _eps_t = nc.alloc_sbuf_tensor("const-float32-eps", [128, 1], F32)
    nc.gpsimd.memset(_eps_t.ap(), 1e-8)
    nc.const_aps.aps[(F32, 1e-8)] = _eps_t.ap()
    nc.all_engine_barrier()

    sig_d = nc.dram_tensor("sig", [n_sig, SIG_ROW], BF16, kind="ExternalInput")
    wt_d = nc.dram_tensor("wt", [128, 2, NBLK, 128], BF16, kind="ExternalInput")
    out_d = nc.dram_tensor("out", [n_sig, 128, nframes], F32, kind="ExternalOutput")

    AF = mybir.ActivationFunctionType

    with _TC(nc) as tc:
        with (
            tc.tile_pool(name="singles", bufs=1) as singles,
            tc.tile_pool(name="psum", bufs=2, space="PSUM") as psum,
            tc.tile_pool(name="post", bufs=3) as post,
            tc.tile_pool(name="outp", bufs=2) as outp,
        ):
            # weights: [taps_i, cplx, block_j, scales]
            wts = singles.tile([128, 2, NBLK, 128], BF16, tag="wts")
            nc.sync.dma_start(wts[:], wt_d[:])

            # shifted-replica tiles, one per signal
            base = sig_d[:]
            shifts = []
            for s in range(n_sig):
                sh = singles.tile([128, U], BF16, tag=f"shift{s}")
                src = bass.AP(
                    tensor=base.tensor,
                    offset=base.offset + s * SIG_ROW,
                    ap=[[1, 128], [1, U]],
                )
                nc.sync.dma_start(sh[:], src)
                shifts.append(sh)

            for s in range(n_sig):
                sh = shifts[s]
                osb = outp.tile([128, nframes], F32, tag="osb")
                for it in range(n_t0):
                    t0 = it * TILE_N
                    ps_re = psum.tile([128, TILE_N], F32, tag="re")
                    ps_im = psum.tile([128, TILE_N], F32, tag="im")
                    for j in range(NBLK):
                        nc.tensor.matmul(
                            ps_re[:],
                            lhsT=wts[:, 0, j, :],
                            rhs=sh[:, t0 + 128 * j : t0 + 128 * j + TILE_N],
                            start=(j == 0),
                            stop=(j == NBLK - 1),
                        )
                    for j in range(NBLK):
                        nc.tensor.matmul(
                            ps_im[:],
                            lhsT=wts[:, 1, j, :],
                            rhs=sh[:, t0 + 128 * j : t0 + 128 * j + TILE_N],
                            start=(j == 0),
                            stop=(j == NBLK - 1),
                        )
                    # walrus here rejects TT with both operands in PSUM, so
                    # evacuate re via copy and square in SBUF; im squares on ACT
                    cre = post.tile([128, TILE_N], BF16, tag="cre")
                    nc.vector.tensor_copy(cre[:], ps_re[:])
                    sq_im = post.tile([128, TILE_N], BF16, tag="sqim")
                    nc.scalar.activation(sq_im[:], ps_im[:], AF.Square)
                    sq_re = post.tile([128, TILE_N], BF16, tag="sqre")
                    nc.vector.tensor_mul(sq_re[:], cre[:], cre[:])
                    ssum = post.tile([128, TILE_N], BF16, tag="ssum")
                    nc.vector.tensor_add(ssum[:], sq_re[:], sq_im[:])
                    # ln(s + 1e-8) in fp16 (bf16 would wreck exp(0.5*u))
                    u = post.tile([128, TILE_N], mybir.dt.float16, tag="u")
                    nc.scalar.activation(u[:], ssum[:], AF.Ln, bias=1e-8)
                    # v = exp(u/2) = sqrt(s + 1e-8)
                    v = post.tile([128, TILE_N], BF16, tag="v")
                    nc.scalar.activation(v[:], u[:], AF.Exp, scale=0.5)
                    # l = ln(1 + v)
                    l = post.tile([128, TILE_N], BF16, tag="l")
                    nc.scalar.activation(l[:], v[:], AF.Ln, bias=1.0)
                    nc.vector.tensor_reduce(
                        osb[:, it * fpt : (it + 1) * fpt],
                        l[:].rearrange("p (f w) -> p f w", w=hop),
                        axis=mybir.AxisListType.X,
                        op=mybir.AluOpType.add,
                    )
                # mean = sum / hop
                nc.scalar.mul(osb[:], osb[:], 1.0 / hop)
                nc.sync.dma_start(out_d[s], osb[:])
    if split_waits:
        _split_sync_waits(nc)
    return nc


def prep_inputs(x, weight_real, weight_imag, hop):
    """Host-side shard/layout prep. Returns per-core input maps."""
    x = np.asarray(x, dtype=np.float32)
    wr = np.asarray(weight_real, dtype=np.float32)
    wi = np.asarray(weight_imag, dtype=np.float32)
    B, C, _ = x.shape

    sig = x.reshape(B * C, T)
    sigpad = np.pad(sig, ((0, 0), (PAD, PAD)), mode="reflect")
    total = (N_CORES - 1) * TCHUNK + SIG_ROW          # 17152
    sigpad = np.pad(sigpad, ((0, 0), (0, total - sigpad.shape[1])))
    sig_bf = sigpad.astype(ml_dtypes.bfloat16)

    # weights -> [taps_i, cplx, block_j, scales] bf16, zero-padded to 768 taps
    wpad = np.zeros((2, 128, KPAD), np.float32)
    wpad[0, :, :KTAPS] = wr[:, 0, :]
    wpad[1, :, :KTAPS] = wi[:, 0, :]
    # (c, s, j, i) -> (i, c, j, s)
    wt_host = np.ascontiguousarray(
        wpad.reshape(2, 128, NBLK, 128).transpose(3, 0, 2, 1)
    ).astype(ml_dtypes.bfloat16)

    in_maps = []
    for c in range(N_CORES):
        chunk = np.ascontiguousarray(sig_bf[:, c * TCHUNK : c * TCHUNK + SIG_ROW])
        in_maps.append({"sig": chunk, "wt": wt_host})
    return in_maps


def prep_wt2(weight_real, weight_imag):
    wr = np.asarray(weight_real, dtype=np.float32)
    wi = np.asarray(weight_imag, dtype=np.float32)
    wpad = np.zeros((2, 128, KPAD), np.float32)
    wpad[0, :, :KTAPS] = wr[:, 0, :]
    wpad[1, :, :KTAPS] = wi[:, 0, :]
    # wt2[i, j, 2s+c] = wpad[c, s, 128j+i]
    w4 = wpad.reshape(2, 128, NBLK, 128)          # (c, s, j, i)
    wt2 = np.ascontiguousarray(w4.transpose(3, 2, 1, 0).reshape(128, NBLK, 256))
    return wt2.astype(ml_dtypes.bfloat16)


def prep_pmat(n_tiles=16, hop=64):
    """Pooling matrices: P[t, it, f] = 1/hop if f == fpt*it + t//hop."""
    fpt = 128 // hop
    nframes = n_tiles * fpt
    P = np.zeros((128, n_tiles, nframes), np.float32)
    for it in range(n_tiles):
        for t in range(128):
            P[t, it, fpt * it + t // hop] = 1.0 / hop
    return P.astype(ml_dtypes.bfloat16)


#  scale-support prefix: block j of 128 taps is needed only by the first S_j
#  scales (supports shrink monotonically with scale index).  Computed from
#  the morlet construction: th_s = int(7639.44/f_s), block j needed iff
#  [381-th, 381+th] overlaps [128j, 128j+128).
S_J = [17, 46, 128, 128, 44, 16]
J_ORDER = [2, 3, 1, 4, 0, 5]          # S_j descending


def build_program_v2(n_sig=NSIG, n_tiles=16, hop=64, act_square_every=5,
                     split_waits=True):
    """Transposed conv: time on PSUM partitions, scales streamed (sparse).

    Per (signal, 128-sample tile): 6 matmuls, lhsT = shift slice
    [128 taps, 128 t], rhs = weights [128 taps, (2cplx, S_j scales)],
    accumulated into psum [128 t, 2, 128].  Streams 758 columns per tile
    instead of 1536 (支持 prefix sparsity).  Postproc: squares split
    ACT/DVE, adds on GPSIMD into s_sig [128, n_tiles*128], then a
    per-signal Ln/Exp/Ln chain (single ACT table set), then pooling as
    16 accumulating matmuls with a [128, 32] pooling matrix ->
    psum [32 frames, 128 scales] -> DRAM [n_sig, 32, 128].
    """
    assert hop == 64, "v2 pooling matrices assume hop=64"
    fpt = 128 // hop                      # frames per 128-tile (2)
    nframes = n_tiles * fpt               # 32
    TW = n_tiles * 128                    # 2048 time samples per signal

    nc = bass.Bass()
    _eps_t = nc.alloc_sbuf_tensor("const-float32-eps", [128, 1], F32)
    nc.gpsimd.memset(_eps_t.ap(), 1e-8)
    nc.const_aps.aps[(F32, 1e-8)] = _eps_t.ap()
    nc.all_engine_barrier()

    sig_d = nc.dram_tensor("sig", [n_sig, SIG_ROW], BF16, kind="ExternalInput")
    # weights interleaved (scale, cplx): col 2s+c, so每 block's rhs and psum
    # writes are contiguous prefixes [0, 2*S_j)
    wt_d = nc.dram_tensor("wt2", [128, NBLK, 256], BF16, kind="ExternalInput")
    pmat_d = nc.dram_tensor("pmat", [128, n_tiles, nframes], BF16,
                            kind="ExternalInput")
    out_d = nc.dram_tensor("out", [n_sig, nframes, 128], F32,
                           kind="ExternalOutput")

    AF = mybir.ActivationFunctionType

    with _TC(nc) as tc:
        with (
            tc.tile_pool(name="singles", bufs=1) as singles,
            tc.tile_pool(name="psum", bufs=4, space="PSUM") as psum,
            tc.tile_pool(name="post", bufs=4) as post,
            tc.tile_pool(name="sigbuf", bufs=2) as sigbuf,
            tc.tile_pool(name="outp", bufs=2) as outp,
        ):
            wts = singles.tile([128, NBLK, 256], BF16, tag="wts")
            nc.sync.dma_start(wts[:], wt_d[:])
            pmat = singles.tile([128, n_tiles, nframes], BF16, tag="pmat")
            nc.sync.dma_start(pmat[:], pmat_d[:])

            base = sig_d[:]
            shifts = []
            for s in range(n_sig):
                sh = singles.tile([128, U], BF16, tag=f"shift{s}")
                src = bass.AP(
                    tensor=base.tensor,
                    offset=base.offset + s * SIG_ROW,
                    ap=[[1, 128], [1, U]],
                )
                nc.sync.dma_start(sh[:], src)
                shifts.append(sh)

            tilectr = 0
            for s in range(n_sig):
                sh = shifts[s]
                s_sig = sigbuf.tile([128, TW], BF16, tag="s_sig")
                for it in range(n_tiles):
                    ps = psum.tile([128, 256], F32, tag="conv")
                    for k, j in enumerate(J_ORDER):
                        sj = S_J[j]
                        nc.tensor.matmul(
                            ps[:, 0 : 2 * sj],
                            lhsT=sh[:, 128 * (it + j) : 128 * (it + j) + 128],
                            rhs=wts[:, j, 0 : 2 * sj],
                            start=(k == 0),
                            stop=(k == NBLK - 1),
                            skip_group_check=True,
                        )
                    # squares: rotate a fraction onto ACT, rest on DVE
                    sqb = post.tile([128, 256], BF16, tag="sqb")
                    if tilectr % act_square_every == 0:
                        nc.scalar.activation(sqb[:], ps[:], AF.Square)
                    else:
                        cb = post.tile([128, 256], BF16, tag="cb")
                        nc.vector.tensor_copy(cb[:], ps[:])
                        nc.vector.tensor_mul(sqb[:], cb[:], cb[:])
                    tilectr += 1
                    sq3 = sqb[:].rearrange("p (s c) -> p s c", c=2)
                    nc.gpsimd.tensor_tensor(
                        s_sig[:, it * 128 : (it + 1) * 128],
                        sq3[:, :, 0],
                        sq3[:, :, 1],
                        mybir.AluOpType.add,
                    )
                # ln/exp/ln chain over the whole signal (one ACT table set)
                u = sigbuf.tile([128, TW], mybir.dt.float16, tag="u")
                nc.scalar.activation(u[:], s_sig[:], AF.Ln, bias=1e-8)
                v = sigbuf.tile([128, TW], BF16, tag="v")
                nc.scalar.activation(v[:], u[:], AF.Exp, scale=0.5)
                l = sigbuf.tile([128, TW], BF16, tag="l")
                nc.scalar.activation(l[:], v[:], AF.Ln, bias=1.0)
                # pooling: 16 accumulating matmuls -> [32 frames, 128 scales]
                pps = psum.tile([nframes, 128], F32, tag="pool", bufs=2)
                for it in range(n_tiles):
                    nc.tensor.matmul(
                        pps[:],
                        lhsT=pmat[:, it, :],
                        rhs=l[:, it * 128 : (it + 1) * 128],
                        start=(it == 0),
                        stop=(it == n_tiles - 1),
                        skip_group_check=True,
                    )
                osb = outp.tile([nframes, 128], F32, tag="osb")
                nc.vector.tensor_copy(osb[:], pps[:])
                nc.sync.dma_start(out_d[s], osb[:])
    if split_waits:
        _split_sync_waits(nc)
    return nc


def build_program_v3(n_sig=NSIG, hop=64, n_a=6, split_waits=True,
                     no_pool_b=False, act_squares=3, interleave=True):
    """Hybrid: half the signals conv'd weights-stationary (v1 layout, MM-stream
    heavy), half signal-stationary with scale-prefix sparsity (v2 layout,
    LDW-stream heavy).  The PE's LDWEIGHTS path (1.2 GHz) and matmul column
    stream (2.4 GHz) are parallel resources; interleaving the two forms
    balances them at ~95us instead of 123us for either alone.
    Postproc for both layouts: squares split ACT/DVE, adds on GPSIMD into
    s_sig [128, 2048] bf16, per-signal Ln/Exp/Ln chain (single table set),
    pooling: DVE grouped reduce (A/scale-major) or PE matmul (B/time-major).
    """
    assert hop == 64
    n_b = n_sig - n_a
    NT128 = TCHUNK // 128                 # 16 128-tiles per signal (B form)
    fpt512 = TILE_N // hop                # 8 frames per 512-tile (A form)
    nframes = TCHUNK // hop               # 32

    nc = bass.Bass()
    _eps_t = nc.alloc_sbuf_tensor("const-float32-eps", [128, 1], F32)
    nc.gpsimd.memset(_eps_t.ap(), 1e-8)
    nc.const_aps.aps[(F32, 1e-8)] = _eps_t.ap()
    nc.all_engine_barrier()

    sig_d = nc.dram_tensor("sig", [n_sig, SIG_ROW], BF16, kind="ExternalInput")
    wt_d = nc.dram_tensor("wt", [128, 2, NBLK, 128], BF16, kind="ExternalInput")
    wt2_d = nc.dram_tensor("wt2", [128, NBLK, 256], BF16, kind="ExternalInput")
    pmat_d = nc.dram_tensor("pmat", [128, NT128, nframes], BF16,
                            kind="ExternalInput")
    outa_d = nc.dram_tensor("outa", [max(n_a, 1), 128, nframes], F32,
                            kind="ExternalOutput")
    outb_d = nc.dram_tensor("outb", [max(n_b, 1), nframes, 128], F32,
                            kind="ExternalOutput")

    AF = mybir.ActivationFunctionType

    with _TC(nc) as tc:
        with (
            tc.tile_pool(name="singles", bufs=1) as singles,
            tc.tile_pool(name="psum", bufs=2, space="PSUM") as psum,
            tc.tile_pool(name="post", bufs=4) as post,
            tc.tile_pool(name="sigbuf", bufs=2) as sigbuf,
            tc.tile_pool(name="outp", bufs=2) as outp,
        ):
            wts = singles.tile([128, 2, NBLK, 128], BF16, tag="wts")
            nc.sync.dma_start(wts[:], wt_d[:])
            wts2 = singles.tile([128, NBLK, 256], BF16, tag="wts2")
            nc.sync.dma_start(wts2[:], wt2_d[:])
            pmat = singles.tile([128, NT128, nframes], BF16, tag="pmat")
            nc.sync.dma_start(pmat[:], pmat_d[:])

            base = sig_d[:]
            shifts = []
            for s in range(n_sig):
                sh = singles.tile([128, U], BF16, tag=f"shift{s}")
                src = bass.AP(
                    tensor=base.tensor,
                    offset=base.offset + s * SIG_ROW,
                    ap=[[1, 128], [1, U]],
                )
                nc.sync.dma_start(sh[:], src)
                shifts.append(sh)

            tilectr = 0

            def conv_a_tile(sh, s_sig, it):
                nonlocal tilectr
                t0 = it * TILE_N
                ps_re = psum.tile([128, TILE_N], F32, tag="are", name="are")
                ps_im = psum.tile([128, TILE_N], F32, tag="aim", name="aim")
                for j in range(NBLK):
                    nc.tensor.matmul(
                        ps_re[:], lhsT=wts[:, 0, j, :],
                        rhs=sh[:, t0 + 128 * j : t0 + 128 * j + TILE_N],
                        start=(j == 0), stop=(j == NBLK - 1),
                        skip_group_check=True,
                    )
                for j in range(NBLK):
                    nc.tensor.matmul(
                        ps_im[:], lhsT=wts[:, 1, j, :],
                        rhs=sh[:, t0 + 128 * j : t0 + 128 * j + TILE_N],
                        start=(j == 0), stop=(j == NBLK - 1),
                        skip_group_check=True,
                    )
                # squares -> s_sig[:, t0:t0+512]
                sq_re = post.tile([128, TILE_N], BF16, tag="asqre", name="asqre")
                if act_squares and tilectr % act_squares == 0:
                    nc.scalar.activation(sq_re[:], ps_re[:], AF.Square)
                else:
                    cre = post.tile([128, TILE_N], BF16, tag="acre", name="acre")
                    nc.vector.tensor_copy(cre[:], ps_re[:])
                    nc.vector.tensor_mul(sq_re[:], cre[:], cre[:])
                sq_im = post.tile([128, TILE_N], BF16, tag="asqim", name="asqim")
                if act_squares and tilectr % act_squares == 1:
                    nc.scalar.activation(sq_im[:], ps_im[:], AF.Square)
                else:
                    cim = post.tile([128, TILE_N], BF16, tag="acim", name="acim")
                    nc.vector.tensor_copy(cim[:], ps_im[:])
                    nc.vector.tensor_mul(sq_im[:], cim[:], cim[:])
                tilectr += 1
                nc.gpsimd.tensor_tensor(
                    s_sig[:, t0 : t0 + TILE_N], sq_re[:], sq_im[:],
                    mybir.AluOpType.add,
                )

            def conv_b_tile(sh, s_sig, it):
                nonlocal tilectr
                ps = psum.tile([128, 256], F32, tag="bconv", name="bconv")
                for k, j in enumerate(J_ORDER):
                    sj = S_J[j]
                    nc.tensor.matmul(
                        ps[:, 0 : 2 * sj],
                        lhsT=sh[:, 128 * (it + j) : 128 * (it + j) + 128],
                        rhs=wts2[:, j, 0 : 2 * sj],
                        start=(k == 0), stop=(k == NBLK - 1),
                        skip_group_check=True,
                    )
                sqb = post.tile([128, 256], BF16, tag="bsqb", name="bsqb")
                if act_squares and tilectr % act_squares == 0:
                    nc.scalar.activation(sqb[:], ps[:], AF.Square)
                else:
                    cb = post.tile([128, 256], BF16, tag="bcb", name="bcb")
                    nc.vector.tensor_copy(cb[:], ps[:])
                    nc.vector.tensor_mul(sqb[:], cb[:], cb[:])
                tilectr += 1
                sq3 = sqb[:].rearrange("p (s c) -> p s c", c=2)
                nc.gpsimd.tensor_tensor(
                    s_sig[:, it * 128 : (it + 1) * 128],
                    sq3[:, :, 0], sq3[:, :, 1], mybir.AluOpType.add,
                )

            def chain(s_sig):
                u = sigbuf.tile([128, TCHUNK], mybir.dt.float16, tag="u", name="u")
                nc.scalar.activation(u[:], s_sig[:], AF.Ln, bias=1e-8)
                v = sigbuf.tile([128, TCHUNK], BF16, tag="v", name="v")
                nc.scalar.activation(v[:], u[:], AF.Exp, scale=0.5)
                l = sigbuf.tile([128, TCHUNK], BF16, tag="l", name="l")
                nc.scalar.activation(l[:], v[:], AF.Ln, bias=1.0)
                return l

            def finish_a(l, sa):
                osb = outp.tile([128, nframes], F32, tag="osba", name="osba")
                nc.vector.tensor_reduce(
                    osb[:], l[:].rearrange("p (f w) -> p f w", w=hop),
                    axis=mybir.AxisListType.X, op=mybir.AluOpType.add,
                )
                nc.scalar.mul(osb[:], osb[:], 1.0 / hop)
                nc.sync.dma_start(outa_d[sa], osb[:])

            def finish_b(l, sb):
                osb = outp.tile([nframes, 128], F32, tag="osbb", name="osbb")
                if no_pool_b:
                    nc.vector.tensor_copy(osb[:], l[:, 0:nframes].rearrange("p f -> p f"))
                    nc.sync.dma_start(outb_d[sb], osb[:].rearrange("p f -> p f"))
                    return
                pps = psum.tile([nframes, 128], F32, tag="bpool", name="bpool")
                for it in range(NT128):
                    nc.tensor.matmul(
                        pps[:], lhsT=pmat[:, it, :],
                        rhs=l[:, it * 128 : (it + 1) * 128],
                        start=(it == 0), stop=(it == NT128 - 1),
                        skip_group_check=True,
                    )
                nc.vector.tensor_copy(osb[:], pps[:])
                nc.sync.dma_start(outb_d[sb], osb[:])

            # interleave A and B signals pairwise so both PE streams stay busy
            npairs = max(n_a, n_b)
            for p in range(npairs):
                sa = p if p < n_a else None
                sb = p if p < n_b else None
                ssa = (
                    sigbuf.tile([128, TCHUNK], BF16, tag="ssa", name="ssa")
                    if sa is not None else None
                )
                ssb = (
                    sigbuf.tile([128, TCHUNK], BF16, tag="ssb", name="ssb")
                    if sb is not None else None
                )
                if interleave:
                    for k in range(NT0):      # 4 super-steps
                        if sa is not None:
                            conv_a_tile(shifts[sa], ssa, k)
                        if sb is not None:
                            for it in range(4 * k, 4 * k + 4):
                                conv_b_tile(shifts[n_a + sb], ssb, it)
                else:
                    if sa is not None:
                        for k in range(NT0):
                            conv_a_tile(shifts[sa], ssa, k)
                    if sb is not None:
                        for it in range(NT128):
                            conv_b_tile(shifts[n_a + sb], ssb, it)
                if sa is not None:
                    finish_a(chain(ssa), sa)
                if sb is not None:
                    finish_b(chain(ssb), sb)
    if split_waits:
        _split_sync_waits(nc)
    return nc



# --------------------------------------------------------------------------
# v5: windowed signal-stationary conv (5-block truncated filter grid),
#     pair-merged PSUM evacuation, GPSIMD pairwise adds, interleaved pooling
# --------------------------------------------------------------------------
B0TAP = 62                       # 5-block grid covers taps [62, 702)
S5 = [29, 73, 128, 73, 28]       # scales needing each 128-tap block (prefixes)
NBLK5 = 5
N_T128 = TCHUNK // 128           # 16 output tiles of 128 samples
NW5 = N_T128 + NBLK5 - 1         # 20 lhsT windows per signal
F16 = mybir.dt.float16


def dedup_ldweights(nc):
    """Drop InstLdweights identical to the previous one in the same block
    (same weights AP/flags, no intervening branch). Sync info of dropped
    instructions is merged onto the next kept instruction."""
    n = 0
    for fn in nc.m.functions:
        for bb in fn.blocks:
            new = []
            last_key = None
            pend_w, pend_u = [], []
            for inst in bb.instructions:
                tn = type(inst).__name__
                if tn == "InstLdweights":
                    key = (str(inst.ins[0]), inst.perf_mode, inst.is_transpose,
                           getattr(inst, "tile_position", None))
                    if key == last_key:
                        si = inst.sync_info
                        if si is not None:
                            pend_w.extend(si.on_wait)
                            pend_u.extend(si.on_update)
                        n += 1
                        continue
                    last_key = key
                elif tn in ("InstCall", "InstUnconditionalBranch"):
                    last_key = None
                if pend_w or pend_u:
                    si = inst.sync_info
                    if si is None:
                        inst.sync_info = mybir.SyncInfo(on_wait=pend_w, on_update=pend_u)
                    else:
                        inst.sync_info = mybir.SyncInfo(
                            on_wait=list(si.on_wait) + pend_w,
                            on_update=list(si.on_update) + pend_u)
                    pend_w, pend_u = [], []
                new.append(inst)
            bb.instructions = new
    return n


def build_program_v5(n_sig=NSIG, hop=64, act_pairs=(2, 5), split_waits=True,
                     dedup_ldw=True):
    """Per-core program, all 12 signals.

    Conv: per signal, 20 windows; window w loads sh[:, 128w+62 : +128] as the
    stationary operand once; blocks k=0..4 accumulate into tile it=w-k's half
    of a 2-bank PSUM pair tile (psum writes [t', 2s+c], prefix width 2*S5[k]).
    Tile it's first contribution (k=0) carries start=True (bank-wide
    has_written clear; each element's first writer overwrites, later ones
    accumulate).

    Postproc per pair (2 tiles, 512 psum f32): ACT pairs: Square direct from
    psum; DVE pairs: copy + self-mul (2x bf16). GPSIMD: strided pairwise adds
    re^2+im^2 -> ssig [128, 2048]. Chain on ACT: Ln(+1e-8), Exp(*0.5),
    Ln(+1) = log1p(sqrt(s)).

    Pooling: 16 matmuls per signal (lhsT = l-tile [t', s], rhs = pool2
    [t', 2] = 1/hop indicator), interleaved one-per-window into the conv of
    signal s+2 so their LDWEIGHTS hide under conv streams. Output [s, 32] f32.
    """
    assert hop == 64
    nframes = TCHUNK // hop          # 32
    fpt = 128 // hop                 # 2 frames per 128-tile

    nc = bass.Bass()
    _eps_t = nc.alloc_sbuf_tensor("const-float32-eps", [128, 1], F32)
    nc.gpsimd.memset(_eps_t.ap(), 1e-8)
    nc.const_aps.aps[(F32, 1e-8)] = _eps_t.ap()
    nc.all_engine_barrier()

    sig_d = nc.dram_tensor("sig", [n_sig, SIG_ROW], BF16, kind="ExternalInput")
    wt5_d = nc.dram_tensor("wt5", [128, NBLK5, 256], BF16, kind="ExternalInput")
    pool2_d = nc.dram_tensor("pool2", [128, fpt], BF16, kind="ExternalInput")
    out_d = nc.dram_tensor("out", [n_sig, 128, nframes], F32, kind="ExternalOutput")

    AF = mybir.ActivationFunctionType

    with _TC(nc) as tc:
        with (
            tc.tile_pool(name="singles", bufs=1) as singles,
            tc.tile_pool(name="psc", bufs=3, space="PSUM") as psc,
            tc.tile_pool(name="pps", bufs=2, space="PSUM") as ppsp,
            tc.tile_pool(name="post", bufs=3) as post,
            tc.tile_pool(name="ssigp", bufs=2) as ssigp,
            tc.tile_pool(name="chainp", bufs=2) as chainp,
            tc.tile_pool(name="lp", bufs=3) as lp,
            tc.tile_pool(name="outp", bufs=2) as outp,
        ):
            wt5 = singles.tile([128, NBLK5, 256], BF16, tag="wt5")
            nc.sync.dma_start(wt5[:], wt5_d[:])
            pool2 = singles.tile([128, fpt], BF16, tag="pool2")
            nc.sync.dma_start(pool2[:], pool2_d[:])

            base = sig_d[:]
            shifts = []
            for s in range(n_sig):
                sh = singles.tile([128, U], BF16, tag=f"shift{s}")
                src = bass.AP(
                    tensor=base.tensor,
                    offset=base.offset + s * SIG_ROW,
                    ap=[[1, 128], [1, U]],
                )
                nc.sync.dma_start(sh[:], src)
                shifts.append(sh)

            l_hist = {}
            pps_hist = {}

            def pool_mm(sp, it2):
                nc.tensor.matmul(
                    pps_hist[sp][:, fpt * it2 : fpt * it2 + fpt],
                    lhsT=l_hist[sp][:, it2 * 128 : (it2 + 1) * 128],
                    rhs=pool2[:],
                    start=True, stop=True, skip_group_check=True,
                )
                if it2 == N_T128 - 1:
                    osb = outp.tile([128, nframes], F32, tag="osb", name=f"osb{sp}")
                    nc.vector.tensor_copy(osb[:], pps_hist[sp][:, 0:nframes])
                    nc.sync.dma_start(out_d[sp], osb[:])
                    del l_hist[sp]
                    del pps_hist[sp]

            def evac_pair(pt, ssig, p, use_act):
                # pair covers ssig cols [256p, 256p+256); psum halves at
                # [0:256] and [512:768] of the 2-bank tile
                pin = pt[:].rearrange("p (h x) -> p h x", h=2)[:, :, 0:256]
                sq = post.tile([128, 512], BF16, tag="sq", name=f"sq{p}")
                if use_act:
                    nc.scalar.activation(
                        sq[:].rearrange("p (h x) -> p h x", h=2), pin, AF.Square)
                else:
                    c = post.tile([128, 512], BF16, tag="c", name=f"c{p}")
                    nc.vector.tensor_copy(
                        c[:].rearrange("p (h x) -> p h x", h=2), pin)
                    nc.vector.tensor_mul(sq[:], c[:], c[:])
                sq4 = sq[:].rearrange("p (h s c) -> p h s c", h=2, c=2)
                nc.gpsimd.tensor_tensor(
                    ssig[:, 256 * p : 256 * p + 256].rearrange(
                        "p (h s) -> p h s", h=2),
                    sq4[:, :, :, 0], sq4[:, :, :, 1], mybir.AluOpType.add,
                )

            for s in range(n_sig):
                ssig = ssigp.tile([128, TCHUNK], BF16, tag="ssig", name=f"ssig{s}")
                pairs = {}
                for w in range(NW5):
                    for k in range(NBLK5):
                        it = w - k
                        if not 0 <= it < N_T128:
                            continue
                        p, half = divmod(it, 2)
                        if k == 0 and half == 0:
                            pairs[p] = psc.tile([128, 1024], F32, tag="pair",
                                                name=f"pair{s}_{p}")
                        nc.tensor.matmul(
                            pairs[p][:, half * 512 : half * 512 + 2 * S5[k]],
                            lhsT=shifts[s][:, 128 * w + B0TAP : 128 * w + B0TAP + 128],
                            rhs=wt5[:, k, 0 : 2 * S5[k]],
                            start=(k == 0), stop=(k == NBLK5 - 1),
                            skip_group_check=True,
                        )
                    if s >= 2 and w >= 4:
                        pool_mm(s - 2, w - 4)
                    if w >= 5 and (w - 5) % 2 == 0:
                        p = (w - 5) // 2
                        evac_pair(pairs.pop(p), ssig, p, p in act_pairs)
                # chain: log1p(sqrt(ssig + 1e-8))
                u = chainp.tile([128, TCHUNK], F16, tag="u", name=f"u{s}")
                nc.scalar.activation(u[:], ssig[:], AF.Ln, bias=1e-8)
                v = chainp.tile([128, TCHUNK], BF16, tag="v", name=f"v{s}")
                nc.scalar.activation(v[:], u[:], AF.Exp, scale=0.5)
                l = lp.tile([128, TCHUNK], BF16, tag="l", name=f"l{s}")
                nc.scalar.activation(l[:], v[:], AF.Ln, bias=1.0)
                l_hist[s] = l
                pps_hist[s] = ppsp.tile([128, 512], F32, tag="pps", name=f"pps{s}")
            # tail: pools for the last two signals
            for sp in (n_sig - 2, n_sig - 1):
                for it2 in range(N_T128):
                    pool_mm(sp, it2)
    if dedup_ldw:
        dedup_ldweights(nc)
    if split_waits:
        _split_sync_waits(nc)
    return nc


def prep_wt5(weight_real, weight_imag):
    wr = np.asarray(weight_real, dtype=np.float32)
    wi = np.asarray(weight_imag, dtype=np.float32)
    wpad = np.zeros((2, 128, KPAD), np.float32)
    wpad[0, :, :KTAPS] = wr[:, 0, :]
    wpad[1, :, :KTAPS] = wi[:, 0, :]
    w5 = wpad[:, :, B0TAP : B0TAP + NBLK5 * 128].reshape(2, 128, NBLK5, 128)
    # wt5[i, k, 2s+c] = wpad[c, s, B0TAP + 128k + i]
    wt5 = np.ascontiguousarray(w5.transpose(3, 2, 1, 0).reshape(128, NBLK5, 256))
    return wt5.astype(ml_dtypes.bfloat16)


def prep_pool2(hop=64):
    fpt = 128 // hop
    p = np.zeros((128, fpt), np.float32)
    for t in range(128):
        p[t, t // hop] = 1.0 / hop
    return p.astype(ml_dtypes.bfloat16)


def _ensure_ntff_hook():
    """Provide antenv.axon_hooks (missing in this image) so trace=True works."""
    import sys as _sys
    import types as _types

    try:
        from antenv.axon_hooks import get_axon_ntff_profile_hook  # noqa: F401
        return
    except ImportError:
        pass
    import antenv
    from trn_agent_boot.trn_boot import _ntff_profile_via_ctypes

    mod = _types.ModuleType("antenv.axon_hooks")
    holder = [None]
    mod.set_axon_ntff_profile_hook = lambda h: holder.__setitem__(0, h)
    mod.get_axon_ntff_profile_hook = lambda: holder[0]
    _sys.modules["antenv.axon_hooks"] = mod
    antenv.axon_hooks = mod
    mod.set_axon_ntff_profile_hook(
        _ntff_profile_via_ctypes("/opt/axon/libaxon_pjrt.so")
    )


_prog_cache = {}


def run(x, weight_real, weight_imag, hop_length, trace=False, trace_kwargs=None,
        version=2):
    """Run the kernel on 8 cores; returns (output, BassKernelResults)."""
    hop = int(hop_length)
    key = (version, hop)
    if key not in _prog_cache:
        if version == 5 and hop == 64:
            _prog_cache[key] = build_program_v5(hop=hop)
        elif version == 3 and hop == 64:
            _prog_cache[key] = build_program_v3(hop=hop)
        elif version == 2 and hop == 64:
            _prog_cache[key] = build_program_v2(hop=hop)
        else:
            key = (1, hop)
            if key not in _prog_cache:
                _prog_cache[key] = build_program(hop=hop)
    nc = _prog_cache[key]
    version = key[0]

    in_maps = prep_inputs(x, weight_real, weight_imag, hop)
    if version == 5:
        wt5 = prep_wt5(weight_real, weight_imag)
        pool2 = prep_pool2(hop)
        for m in in_maps:
            m["wt5"] = wt5
            m["pool2"] = pool2
            del m["wt"]
    if version in (2, 3):
        pmat = prep_pmat()
        wt2 = prep_wt2(weight_real, weight_imag)
        for m in in_maps:
            m["pmat"] = pmat
            m["wt2"] = wt2
            if version == 2:
                del m["wt"]
    kwargs = {}
    if trace:
        _ensure_ntff_hook()
        kwargs["trace"] = True
        kwargs.update(trace_kwargs or {})
    res = run_bass_kernel_spmd(nc, in_maps, core_ids=list(range(N_CORES)), **kwargs)

    B, C = 4, 3
    nf_core = TCHUNK // hop
    N_A = 6
    out = np.empty((NSIG, 128, N_CORES * nf_core), np.float32)
    for c in range(N_CORES):
        sl = slice(c * nf_core, (c + 1) * nf_core)
        if version == 5:
            out[:, :, sl] = res.results[c]["out"]
        elif version == 3:
            out[:N_A, :, sl] = res.results[c]["outa"]
            out[N_A:, :, sl] = res.results[c]["outb"].transpose(0, 2, 1)
        elif version == 2:
            out[:, :, sl] = res.results[c]["out"].transpose(0, 2, 1)
        else:
            out[:, :, sl] = res.results[c]["out"]
    return out.reshape(B, C, 128, N_CORES * nf_core), res


def kernel(x, weight_real, weight_imag, hop_length):
    out, _ = run(x, weight_real, weight_imag, hop_length,
                 version=5 if int(hop_length) == 64 else 1)
    return out

